# revision 15
# baseline (speedup 1.0000x reference)
"""Trainium2 Bass kernel for nn_AttentionLayer_19782619365684.

Computes, for h[32,1024], v[32,2048,512], W1[512,1024], b1[512], W2[512,512],
b2[512], w3[512]:
    hp = h @ W1.T + b1                      # [B, P]
    vp = einsum('bfp,qp->bfq', v, W2) + b2  # [B, F, P]
    e  = einsum('bfp,p->bf', tanh(hp[:,None,:] + vp), w3)
    a  = softmax(e.T.reshape(-1).reshape(B, F), axis=1)

Strategy (8 NeuronCores, data parallel over frames, zero communication):
  - Shard F=2048 frames -> 256 per core. Scrambled output row i needs
    e[b, f] for f in [64i, 64(i+1)) over all b, so core c (frames
    [256c, 256c+256)) owns exactly output rows [4c, 4c+4).
  - Rows within a core are ordered f-major (r = f_local*32 + b). In that
    order the scramble flatten is the identity: group g of 512 rows is
    columns [512g, 512g+512) of the row-major scrambled stream.
  - v is pre-transposed on the host to [sg, pc, 128, rows] so each
    super-group loads with ONE 4 MiB DMA and the contraction dim sits on
    SBUF partitions; matmuls run as f32r (full PE rate at N=512).
  - The tanh bias (hp[b,:] + b1 + b2) is folded into the PSUM
    accumulation itself: a K=33 matmul against a constant one-hot
    pattern (rows 0-31 select b = r%32, row 32 = ones carrying b1+b2)
    seeds PSUM, then the four K=128 W2 chunks accumulate on top. ACT
    reads PSUM directly for tanh - the vector engine stays nearly idle,
    so the PE never stalls long enough for HAM to re-throttle.
  - The w3 dot is a skinny M=4 f32r... bf16 matmul pipelined one group
    behind; exp runs off PSUM partition 0 with a fused accumulated row
    sum, and the final scale is split DVE/ACT.
"""

import os
import sys

import numpy as np

for _p in ("/opt/trn_rl_repo", "/root/.axon_site/_ro/trn_rl_repo"):
    if os.path.isdir(_p) and _p not in sys.path:
        sys.path.insert(0, _p)

import concourse.bacc as bacc
import concourse.bass as bass
import concourse.tile as tile
from concourse import mybir
from concourse.bass_utils import run_bass_kernel_spmd

B = 32          # batch
F = 2048        # num frames (global)
H = 1024        # h hidden dim
P = 512         # v feature dim / W2 dim
NCORES = 8
FL = F // NCORES            # frames per core = 256
R = B * FL                  # rows per core = 8192
GR = 512                    # rows per compute group
NG = R // GR                # compute groups = 16
SG_ROWS = 2048              # rows per DMA super-group
NSG = R // SG_ROWS          # super-groups = 4
GPSG = SG_ROWS // GR        # compute groups per super-group = 4
QC = P // 128               # q chunks = 4
PC = P // 128               # p chunks = 4
KC = H // 128               # k chunks for the hp matmul = 8

F32 = mybir.dt.float32
F32R = mybir.dt.float32r
BF16 = mybir.dt.bfloat16
AF = mybir.ActivationFunctionType

TRACE = False           # set True (from test.py) to capture an NTFF profile
LAST_RESULTS = None     # BassKernelResults of the most recent run


def build_nc():
    nc = bacc.Bacc("TRN2", target_bir_lowering=False)

    vt = nc.declare_dram_parameter("vt", [NSG, PC, 128, SG_ROWS], F32R,
                                   isOutput=False)[:]
    w2t = nc.declare_dram_parameter("w2t", [P, P], F32R, isOutput=False)[:]
    w1t = nc.declare_dram_parameter("w1t", [H, P], F32R, isOutput=False)[:]
    ht = nc.declare_dram_parameter("ht", [H, B], F32R, isOutput=False)[:]
    b1r = nc.declare_dram_parameter("b1r", [1, P], F32, isOutput=False)[:]
    b2r = nc.declare_dram_parameter("b2r", [1, P], F32, isOutput=False)[:]
    w3r = nc.declare_dram_parameter("w3r", [128, QC, 4], BF16, isOutput=False)[:]
    oh = nc.declare_dram_parameter("oh", [B + 1, GR], BF16, isOutput=False)[:]
    out = nc.declare_dram_parameter("out", [4, F], F32, isOutput=True)[:]

    with tile.TileContext(nc) as tc:
        with (
            tc.tile_pool(name="singles", bufs=1) as singles,
            tc.tile_pool(name="vt_pool", bufs=2) as vtp,
            tc.tile_pool(name="x_pool", bufs=2) as xp,
            tc.tile_pool(name="vp_psum", bufs=3, space="PSUM") as vpp,
            tc.tile_pool(name="e_psum", bufs=2, space="PSUM") as epp,
        ):
            # ---- one-time loads ----
            w2t_sb = singles.tile([128, PC, P], F32R)
            nc.sync.dma_start(w2t_sb[:], w2t.rearrange("(po pi) q -> pi po q", pi=128))
            w1t_sb = singles.tile([128, KC, P], F32R)
            nc.sync.dma_start(w1t_sb[:], w1t.rearrange("(ko ki) q -> ki ko q", ki=128))
            ht_sb = singles.tile([128, KC, B], F32R)
            nc.sync.dma_start(ht_sb[:], ht.rearrange("(ko ki) b -> ki ko b", ki=128))
            b1_sb = singles.tile([1, P], F32)
            nc.sync.dma_start(b1_sb[:], b1r)
            b2_sb = singles.tile([1, P], F32)
            nc.sync.dma_start(b2_sb[:], b2r)
            w3_sb = singles.tile([128, QC, 4], BF16)
            nc.sync.dma_start(w3_sb[:], w3r)
            oh_sb = singles.tile([B + 1, GR], BF16)
            nc.sync.dma_start(oh_sb[:], oh)

            # ---- hpb_aug: rows 0-31 = hp[b, :] = (h @ W1.T)[b, :],
            #      row 32 = b1 + b2 (combined with the one-hot ones row) ----
            hpb_aug = singles.tile([B + 1, P], BF16)
            hp_ps = vpp.tile([128, 2, GR], F32, tag="vp")
            for kc in range(KC):
                nc.tensor.matmul(
                    hp_ps[:B, 0, :],
                    lhsT=ht_sb[:, kc, :],
                    rhs=w1t_sb[:, kc, :],
                    start=(kc == 0),
                    stop=(kc == KC - 1),
                )
            nc.vector.tensor_copy(hpb_aug[0:B, :], hp_ps[:B, 0, :])
            # b1+b2 row lives on partition 32; DVE can't shift partitions,
            # so add on partition 0 then move via a casting SWDGE DMA.
            b12_sb = singles.tile([1, P], F32)
            nc.vector.tensor_add(b12_sb[:], b1_sb[:], b2_sb[:])
            nc.gpsimd.dma_start(out=hpb_aug[B : B + 1, :], in_=b12_sb[:])

            # exp(e) in scrambled flat order: group g occupies columns
            # [512g, 512g+512), all on partition 0 (PSUM 1-partition reads
            # are only legal at partition 0).
            scram = singles.tile([1, R], F32)
            gsum = singles.tile([1, NG], F32)   # per-group partial row sums

            def w3_stage(x_tile, g):
                # e = w3 . x, contracting q on partitions; lhsT is w3
                # replicated to M=4 columns, partition 0 of PSUM holds e.
                e_ps = epp.tile([4, GR], F32)
                for qc in range(QC):
                    nc.tensor.matmul(
                        e_ps[:],
                        lhsT=w3_sb[:, qc, :],
                        rhs=x_tile[:, qc, :],
                        start=(qc == 0),
                        stop=(qc == QC - 1),
                    )
                nc.scalar.activation(
                    scram[0:1, GR * g : GR * (g + 1)],
                    e_ps[0:1, :],
                    AF.Exp,
                    accum_out=gsum[0:1, g : g + 1],
                )

            pend = None
            for sg in range(NSG):
                vt_sb = vtp.tile([128, PC, SG_ROWS], F32R)
                nc.sync.dma_start(vt_sb[:], vt[sg].rearrange("pc pi f -> pi pc f"))
                for lg in range(GPSG):
                    g = sg * GPSG + lg
                    x = xp.tile([128, QC, GR], BF16)
                    for half in range(2):
                        vp = vpp.tile([128, 2, GR], F32, tag="vp")
                        for qh in range(2):
                            qc = 2 * half + qh
                            # bias seed: PSUM[q, r] = hp[r%32, q] + b1[q] + b2[q]
                            nc.tensor.matmul(
                                vp[:, qh, :],
                                lhsT=hpb_aug[:, 128 * qc : 128 * (qc + 1)],
                                rhs=oh_sb[:],
                                start=True,
                                stop=False,
                            )
                            for pc in range(PC):
                                nc.tensor.matmul(
                                    vp[:, qh, :],
                                    lhsT=w2t_sb[:, pc, 128 * qc : 128 * (qc + 1)],
                                    rhs=vt_sb[:, pc, GR * lg : GR * (lg + 1)],
                                    start=False,
                                    stop=(pc == PC - 1),
                                )
                        nc.scalar.activation(
                            x[:, 2 * half : 2 * half + 2, :], vp[:], AF.Tanh
                        )
                    if pend is not None:
                        w3_stage(*pend)
                    pend = (x, g)
            w3_stage(*pend)

            # ---- softmax tail: scale by reciprocal row sums, write out ----
            # Output row i covers groups [4i, 4i+4); split the scaling of
            # the four rows across DVE and ACT so they run in parallel.
            stot = singles.tile([1, 4], F32)
            for i in range(4):
                nc.vector.reduce_sum(
                    stot[0:1, i : i + 1], gsum[0:1, 4 * i : 4 * i + 4],
                    axis=mybir.AxisListType.X,
                )
            rinv = singles.tile([1, 4], F32)
            nc.vector.reciprocal(rinv[:], stot[:])
            for i in range(4):
                sl = scram[0:1, F * i : F * (i + 1)]
                if i < 2:
                    nc.vector.tensor_scalar_mul(sl, sl, scalar1=rinv[0:1, i : i + 1])
                else:
                    nc.scalar.mul(sl, sl, mul=rinv[0:1, i : i + 1])
            nc.sync.dma_start(out.rearrange("r f -> (r f)"), scram[0:1, :])

    nc.compile()
    return nc


def make_in_maps(inputs):
    import ml_dtypes

    h = np.asarray(inputs["h"], dtype=np.float32)
    v = np.asarray(inputs["v"], dtype=np.float32)
    W1 = np.asarray(inputs["W1"], dtype=np.float32)
    b1 = np.asarray(inputs["b1"], dtype=np.float32)
    W2 = np.asarray(inputs["W2"], dtype=np.float32)
    b2 = np.asarray(inputs["b2"], dtype=np.float32)
    w3 = np.asarray(inputs["w3"], dtype=np.float32)

    ht = np.ascontiguousarray(h.T)                       # [H, B]
    w1t = np.ascontiguousarray(W1.T)                     # [H, P]
    w2t = np.ascontiguousarray(W2.T)                     # [P, P]
    b1r = np.ascontiguousarray(b1.reshape(1, P))
    b2r = np.ascontiguousarray(b2.reshape(1, P))
    w3r = np.ascontiguousarray(
        np.broadcast_to(w3.reshape(QC, 128).T[:, :, None], (128, QC, 4))
    ).astype(ml_dtypes.bfloat16)

    # one-hot bias-selection pattern: oh[b, r] = (r % 32 == b), oh[32, :] = 1
    ohm = np.zeros((B + 1, GR), np.float32)
    ohm[np.arange(GR) % B, np.arange(GR)] = 1.0
    ohm[B, :] = 1.0
    ohm = ohm.astype(ml_dtypes.bfloat16)

    in_maps = []
    for c in range(NCORES):
        vs = v[:, c * FL : (c + 1) * FL, :]              # [B, FL, P]
        # [P, FL, B] -> row index r = f_local*B + b (f-major), then chunked
        # as [sg, pc, 128, SG_ROWS] so one DMA loads a whole super-group.
        vtc = vs.transpose(2, 1, 0).reshape(PC, 128, NSG, SG_ROWS)
        vtc = np.ascontiguousarray(vtc.transpose(2, 0, 1, 3))
        in_maps.append(
            {"vt": vtc, "w2t": w2t, "w1t": w1t, "ht": ht,
             "b1r": b1r, "b2r": b2r, "w3r": w3r, "oh": ohm}
        )
    return in_maps


_NC_CACHE = None


def kernel(**inputs) -> np.ndarray:
    global _NC_CACHE, LAST_RESULTS
    if _NC_CACHE is None:
        _NC_CACHE = build_nc()
    nc = _NC_CACHE
    in_maps = make_in_maps(inputs)
    res = run_bass_kernel_spmd(nc, in_maps, core_ids=list(range(NCORES)),
                               trace=TRACE)
    LAST_RESULTS = res
    outs = [np.asarray(res.results[c]["out"]) for c in range(NCORES)]
    return np.concatenate(outs, axis=0).astype(np.float32)  # [B, F]


# revision 17
# speedup vs baseline: 1.1073x; 1.1073x over previous
"""Trainium2 Bass kernel for nn_AttentionLayer_19782619365684.

Computes, for h[32,1024], v[32,2048,512], W1[512,1024], b1[512], W2[512,512],
b2[512], w3[512]:
    hp = h @ W1.T + b1                      # [B, P]
    vp = einsum('bfp,qp->bfq', v, W2) + b2  # [B, F, P]
    e  = einsum('bfp,p->bf', tanh(hp[:,None,:] + vp), w3)
    a  = softmax(e.T.reshape(-1).reshape(B, F), axis=1)

Strategy (8 NeuronCores, data parallel over frames, zero communication):
  - Shard F=2048 frames -> 256 per core. Scrambled output row i needs
    e[b, f] for f in [64i, 64(i+1)) over all b, so core c (frames
    [256c, 256c+256)) owns exactly output rows [4c, 4c+4).
  - Rows within a core are ordered f-major (r = f_local*32 + b). In that
    order the scramble flatten is the identity: group g of GR rows is
    columns [GR*g, GR*(g+1)) of the row-major scrambled stream.
  - v is cast to bf16 and pre-transposed on the host to
    [sg, pc, 128, rows]: one 2 MiB DMA per super-group, contraction dim
    on SBUF partitions, bf16 matmuls at full PE rate with N=1024 moving
    operands (and FWL weight loads).
  - The tanh bias (hp[b,:] + b1 + b2) is folded into the PSUM
    accumulation: a K=33 matmul against a constant one-hot pattern
    (rows 0-31 select b = r%32, row 32 = ones carrying b1+b2) seeds
    PSUM, then the four K=128 W2 chunks accumulate on top. ACT reads
    PSUM directly for tanh; the vector engine stays nearly idle.
  - hp itself runs in f32r, with a few redundant warm-up repetitions
    emitted while the first v chunk streams in so the PE's HAM clock
    gate reaches 2.4 GHz before the main matmuls start.
  - The w3 dot is a skinny M=4 bf16 matmul pipelined one group behind;
    exp runs off PSUM partition 0 with a fused accumulated row sum, and
    the final softmax scale is split across DVE and ACT.
"""

import os
import sys

import numpy as np

for _p in ("/opt/trn_rl_repo", "/root/.axon_site/_ro/trn_rl_repo"):
    if os.path.isdir(_p) and _p not in sys.path:
        sys.path.insert(0, _p)

import concourse.bacc as bacc
import concourse.bass as bass
import concourse.tile as tile
from concourse import mybir
from concourse.bass_utils import run_bass_kernel_spmd

B = 32          # batch
F = 2048        # num frames (global)
H = 1024        # h hidden dim
P = 512         # v feature dim / W2 dim
NCORES = 8
FL = F // NCORES            # frames per core = 256
R = B * FL                  # rows per core = 8192
GR = 512                    # rows per compute group (one PSUM bank)
NG = R // GR                # compute groups = 16
SG_ROWS = 2048              # rows per DMA super-group
NSG = R // SG_ROWS          # super-groups = 4
GPSG = SG_ROWS // GR        # compute groups per super-group = 4
QC = P // 128               # q chunks = 4
PC = P // 128               # p chunks = 4
KC = H // 128               # k chunks for the hp matmul = 8
WARMUP_ITERS = 3            # redundant hp repeats to keep the PE warm

F32 = mybir.dt.float32
F32R = mybir.dt.float32r
BF16 = mybir.dt.bfloat16
AF = mybir.ActivationFunctionType

TRACE = False           # set True (from test.py) to capture an NTFF profile
LAST_RESULTS = None     # BassKernelResults of the most recent run


def build_nc():
    nc = bacc.Bacc("TRN2", target_bir_lowering=False)

    vt = nc.declare_dram_parameter("vt", [NSG, PC, 128, SG_ROWS], BF16,
                                   isOutput=False)[:]
    w2t = nc.declare_dram_parameter("w2t", [P, P], BF16, isOutput=False)[:]
    w1t = nc.declare_dram_parameter("w1t", [H, P], F32R, isOutput=False)[:]
    ht = nc.declare_dram_parameter("ht", [H, B], F32R, isOutput=False)[:]
    b1r = nc.declare_dram_parameter("b1r", [1, P], F32, isOutput=False)[:]
    b2r = nc.declare_dram_parameter("b2r", [1, P], F32, isOutput=False)[:]
    w3r = nc.declare_dram_parameter("w3r", [128, QC, 4], BF16, isOutput=False)[:]
    oh = nc.declare_dram_parameter("oh", [B + 1, GR], BF16, isOutput=False)[:]
    out = nc.declare_dram_parameter("out", [4, F], F32, isOutput=True)[:]

    with tile.TileContext(nc) as tc:
        with (
            tc.tile_pool(name="singles", bufs=1) as singles,
            tc.tile_pool(name="vt_pool", bufs=3) as vtp,
            tc.tile_pool(name="x_pool", bufs=2) as xp,
            tc.tile_pool(name="vp_psum", bufs=2, space="PSUM") as vpp,
            tc.tile_pool(name="e_psum", bufs=2, space="PSUM") as epp,
        ):
            # ---- one-time loads (hp dependencies first) ----
            ht_sb = singles.tile([128, KC, B], F32R)
            nc.sync.dma_start(ht_sb[:], ht.rearrange("(ko ki) b -> ki ko b", ki=128))
            w1t_sb = singles.tile([128, KC, P], F32R)
            nc.sync.dma_start(w1t_sb[:], w1t.rearrange("(ko ki) q -> ki ko q", ki=128))
            w2t_sb = singles.tile([128, PC, P], BF16)
            nc.sync.dma_start(w2t_sb[:], w2t.rearrange("(po pi) q -> pi po q", pi=128))
            b1_sb = singles.tile([1, P], F32)
            nc.sync.dma_start(b1_sb[:], b1r)
            b2_sb = singles.tile([1, P], F32)
            nc.sync.dma_start(b2_sb[:], b2r)
            w3_sb = singles.tile([128, QC, 4], BF16)
            nc.sync.dma_start(w3_sb[:], w3r)
            oh_sb = singles.tile([B + 1, GR], BF16)
            nc.sync.dma_start(oh_sb[:], oh)

            # ---- hpb_aug: rows 0-31 = hp[b, :] = (h @ W1.T)[b, :],
            #      row 32 = b1 + b2 (pairs with the one-hot ones row) ----
            hpb_aug = singles.tile([B + 1, P], BF16)
            hp_ps = vpp.tile([128, GR], F32, tag="vp")
            for kc in range(KC):
                nc.tensor.matmul(
                    hp_ps[:B, 0:P],
                    lhsT=ht_sb[:, kc, :],
                    rhs=w1t_sb[:, kc, :],
                    start=(kc == 0),
                    stop=(kc == KC - 1),
                )
            nc.vector.tensor_copy(hpb_aug[0:B, :], hp_ps[:B, 0:P])
            # b1+b2 row lives on partition 32; DVE can't shift partitions,
            # so add on partition 0 then move via a casting SWDGE DMA.
            b12_sb = singles.tile([1, P], F32)
            nc.vector.tensor_add(b12_sb[:], b1_sb[:], b2_sb[:])
            nc.gpsimd.dma_start(out=hpb_aug[B : B + 1, :], in_=b12_sb[:])

            # PE warm-up: redundant hp repetitions into the other PSUM bank
            # keep the HAM activity window busy while the first v chunk
            # streams in, so the main matmuls start at 2.4 GHz.
            wu_ps = vpp.tile([128, GR], F32, tag="vp")
            for it in range(WARMUP_ITERS):
                for kc in range(KC):
                    nc.tensor.matmul(
                        wu_ps[:B, 0:P],
                        lhsT=ht_sb[:, kc, :],
                        rhs=w1t_sb[:, kc, :],
                        start=(kc == 0),
                        stop=(kc == KC - 1),
                    )

            # exp(e) in scrambled flat order: group g occupies columns
            # [GR*g, GR*(g+1)), all on partition 0 (PSUM 1-partition reads
            # are only legal at partition 0).
            scram = singles.tile([1, R], F32)
            gsum = singles.tile([1, NG], F32)   # per-group partial row sums

            def w3_stage(x_tile, g):
                # e = w3 . x, contracting q on partitions; lhsT is w3
                # replicated to M=4 columns, partition 0 of PSUM holds e.
                e_ps = epp.tile([4, GR], F32)
                for qc in range(QC):
                    nc.tensor.matmul(
                        e_ps[:],
                        lhsT=w3_sb[:, qc, :],
                        rhs=x_tile[:, qc, :],
                        start=(qc == 0),
                        stop=(qc == QC - 1),
                    )
                nc.scalar.activation(
                    scram[0:1, GR * g : GR * (g + 1)],
                    e_ps[0:1, :],
                    AF.Exp,
                    accum_out=gsum[0:1, g : g + 1],
                )

            pend = None
            for sg in range(NSG):
                vt_sb = vtp.tile([128, PC, SG_ROWS], BF16)
                nc.sync.dma_start(vt_sb[:], vt[sg].rearrange("pc pi f -> pi pc f"))
                for lg in range(GPSG):
                    g = sg * GPSG + lg
                    x = xp.tile([128, QC, GR], BF16)
                    for qc in range(QC):
                        vp = vpp.tile([128, GR], F32, tag="vp")
                        # bias seed: PSUM[q, r] = hp[r%32, q] + b1[q] + b2[q]
                        nc.tensor.matmul(
                            vp[:],
                            lhsT=hpb_aug[:, 128 * qc : 128 * (qc + 1)],
                            rhs=oh_sb[:],
                            start=True,
                            stop=False,
                        )
                        for pc in range(PC):
                            nc.tensor.matmul(
                                vp[:],
                                lhsT=w2t_sb[:, pc, 128 * qc : 128 * (qc + 1)],
                                rhs=vt_sb[:, pc, GR * lg : GR * (lg + 1)],
                                start=False,
                                stop=(pc == PC - 1),
                            )
                        nc.scalar.activation(x[:, qc, :], vp[:], AF.Tanh)
                    if pend is not None:
                        w3_stage(*pend)
                    pend = (x, g)
            w3_stage(*pend)

            # ---- softmax tail: scale by reciprocal row sums, write out ----
            # Output row i covers groups [4i, 4i+4); split the scaling of
            # the four rows across DVE and ACT so they run in parallel.
            stot = singles.tile([1, 4], F32)
            for i in range(4):
                nc.vector.reduce_sum(
                    stot[0:1, i : i + 1], gsum[0:1, 4 * i : 4 * i + 4],
                    axis=mybir.AxisListType.X,
                )
            rinv = singles.tile([1, 4], F32)
            nc.vector.reciprocal(rinv[:], stot[:])
            for i in range(4):
                sl = scram[0:1, F * i : F * (i + 1)]
                if i < 2:
                    nc.vector.tensor_scalar_mul(sl, sl, scalar1=rinv[0:1, i : i + 1])
                else:
                    nc.scalar.mul(sl, sl, mul=rinv[0:1, i : i + 1])
            nc.sync.dma_start(out.rearrange("r f -> (r f)"), scram[0:1, :])

    nc.compile()
    return nc


def make_in_maps(inputs):
    import ml_dtypes

    h = np.asarray(inputs["h"], dtype=np.float32)
    v = np.asarray(inputs["v"], dtype=np.float32)
    W1 = np.asarray(inputs["W1"], dtype=np.float32)
    b1 = np.asarray(inputs["b1"], dtype=np.float32)
    W2 = np.asarray(inputs["W2"], dtype=np.float32)
    b2 = np.asarray(inputs["b2"], dtype=np.float32)
    w3 = np.asarray(inputs["w3"], dtype=np.float32)

    ht = np.ascontiguousarray(h.T)                       # [H, B]
    w1t = np.ascontiguousarray(W1.T)                     # [H, P]
    w2t = np.ascontiguousarray(W2.T).astype(ml_dtypes.bfloat16)
    b1r = np.ascontiguousarray(b1.reshape(1, P))
    b2r = np.ascontiguousarray(b2.reshape(1, P))
    w3r = np.ascontiguousarray(
        np.broadcast_to(w3.reshape(QC, 128).T[:, :, None], (128, QC, 4))
    ).astype(ml_dtypes.bfloat16)

    # one-hot bias-selection pattern: oh[b, r] = (r % 32 == b), oh[32, :] = 1
    ohm = np.zeros((B + 1, GR), np.float32)
    ohm[np.arange(GR) % B, np.arange(GR)] = 1.0
    ohm[B, :] = 1.0
    ohm = ohm.astype(ml_dtypes.bfloat16)

    in_maps = []
    for c in range(NCORES):
        vs = v[:, c * FL : (c + 1) * FL, :]              # [B, FL, P]
        # [P, FL, B] -> row index r = f_local*B + b (f-major), then chunked
        # as [sg, pc, 128, SG_ROWS] so one DMA loads a whole super-group.
        vtc = vs.transpose(2, 1, 0).reshape(PC, 128, NSG, SG_ROWS)
        vtc = np.ascontiguousarray(vtc.transpose(2, 0, 1, 3)).astype(
            ml_dtypes.bfloat16
        )
        in_maps.append(
            {"vt": vtc, "w2t": w2t, "w1t": w1t, "ht": ht,
             "b1r": b1r, "b2r": b2r, "w3r": w3r, "oh": ohm}
        )
    return in_maps


_NC_CACHE = None


def kernel(**inputs) -> np.ndarray:
    global _NC_CACHE, LAST_RESULTS
    if _NC_CACHE is None:
        _NC_CACHE = build_nc()
    nc = _NC_CACHE
    in_maps = make_in_maps(inputs)
    res = run_bass_kernel_spmd(nc, in_maps, core_ids=list(range(NCORES)),
                               trace=TRACE)
    LAST_RESULTS = res
    outs = [np.asarray(res.results[c]["out"]) for c in range(NCORES)]
    return np.concatenate(outs, axis=0).astype(np.float32)  # [B, F]


# revision 20
# speedup vs baseline: 1.6669x; 1.5053x over previous
"""Trainium2 Bass kernel for nn_AttentionLayer_19782619365684.

Computes, for h[32,1024], v[32,2048,512], W1[512,1024], b1[512], W2[512,512],
b2[512], w3[512]:
    hp = h @ W1.T + b1                      # [B, P]
    vp = einsum('bfp,qp->bfq', v, W2) + b2  # [B, F, P]
    e  = einsum('bfp,p->bf', tanh(hp[:,None,:] + vp), w3)
    a  = softmax(e.T.reshape(-1).reshape(B, F), axis=1)

Strategy (8 NeuronCores, data parallel over frames, zero communication):
  - Shard F=2048 frames -> 256 per core. Scrambled output row i needs
    e[b, f] for f in [64i, 64(i+1)) over all b, so core c (frames
    [256c, 256c+256)) owns exactly output rows [4c, 4c+4).
  - Rows within a core are ordered f-major (r = f_local*32 + b). In that
    order the scramble flatten is the identity: group g of GR rows is
    columns [GR*g, GR*(g+1)) of the row-major scrambled stream.
  - v is cast to bf16 and pre-transposed on the host to
    [sg, pc, 128, rows]: one 2 MiB DMA per super-group, contraction dim
    on SBUF partitions, bf16 matmuls at full PE rate with FWL loads.
  - PE is the bottleneck engine, so the tanh bias (hp[b,:] + b1 + b2,
    replicated along the row axis with period 32) is added by the
    otherwise-idle vector engine, and the w3 dot is a skinny M=4 bf16
    matmul pipelined one group behind.
  - exp runs off PSUM partition 0 with a fused accumulated row sum; each
    output row is scaled and written out as soon as its last group's
    exp lands, so the tail does not serialize after the last matmul.
"""

import os
import sys

import numpy as np

for _p in ("/opt/trn_rl_repo", "/root/.axon_site/_ro/trn_rl_repo"):
    if os.path.isdir(_p) and _p not in sys.path:
        sys.path.insert(0, _p)

import concourse.bacc as bacc
import concourse.bass as bass
import concourse.tile as tile
from concourse import mybir
from concourse.bass_utils import run_bass_kernel_spmd

B = 32          # batch
F = 2048        # num frames (global)
H = 1024        # h hidden dim
P = 512         # v feature dim / W2 dim
NCORES = 8
FL = F // NCORES            # frames per core = 256
R = B * FL                  # rows per core = 8192
GR = 512                    # rows per compute group (one PSUM bank)
NG = R // GR                # compute groups = 16
SG_ROWS = 2048              # rows per DMA super-group
NSG = R // SG_ROWS          # super-groups = 4
GPSG = SG_ROWS // GR        # compute groups per super-group = 4
QC = P // 128               # q chunks = 4
PC = P // 128               # p chunks = 4
KC = H // 128               # k chunks for the hp matmul = 8
WARMUP_ITERS = 1            # redundant hp repeats to keep the PE warm

F32 = mybir.dt.float32
F32R = mybir.dt.float32r
BF16 = mybir.dt.bfloat16
AF = mybir.ActivationFunctionType

TRACE = False           # set True (from test.py) to capture an NTFF profile
LAST_RESULTS = None     # BassKernelResults of the most recent run


def build_nc():
    nc = bacc.Bacc("TRN2", target_bir_lowering=False)

    vt = nc.declare_dram_parameter("vt", [NSG, PC, 128, SG_ROWS], BF16,
                                   isOutput=False)[:]
    w2t = nc.declare_dram_parameter("w2t", [P, P], BF16, isOutput=False)[:]
    w1t = nc.declare_dram_parameter("w1t", [H, P], BF16, isOutput=False)[:]
    ht = nc.declare_dram_parameter("ht", [H, B], BF16, isOutput=False)[:]
    b1r = nc.declare_dram_parameter("b1r", [128, QC], F32, isOutput=False)[:]
    b2r = nc.declare_dram_parameter("b2r", [128, QC], F32, isOutput=False)[:]
    w3r = nc.declare_dram_parameter("w3r", [128, QC, 4], BF16, isOutput=False)[:]
    out = nc.declare_dram_parameter("out", [4, F], F32, isOutput=True)[:]

    with tile.TileContext(nc) as tc:
        with (
            tc.tile_pool(name="singles", bufs=1) as singles,
            tc.tile_pool(name="vt_pool", bufs=3) as vtp,
            tc.tile_pool(name="z_pool", bufs=2) as zp,
            tc.tile_pool(name="x_pool", bufs=2) as xp,
            tc.tile_pool(name="vp_psum", bufs=4, space="PSUM") as vpp,
            tc.tile_pool(name="e_psum", bufs=2, space="PSUM") as epp,
            tc.tile_pool(name="hp_psum", bufs=2, space="PSUM") as hpp,
        ):
            # ---- one-time loads (hp dependencies first) ----
            ht_sb = singles.tile([128, KC, B], BF16)
            nc.sync.dma_start(ht_sb[:], ht.rearrange("(ko ki) b -> ki ko b", ki=128))
            w1t_sb = singles.tile([128, KC, P], BF16)
            nc.sync.dma_start(w1t_sb[:], w1t.rearrange("(ko ki) q -> ki ko q", ki=128))
            w2t_sb = singles.tile([128, PC, P], BF16)
            nc.sync.dma_start(w2t_sb[:], w2t.rearrange("(po pi) q -> pi po q", pi=128))
            b1_sb = singles.tile([128, QC], F32)
            nc.sync.dma_start(b1_sb[:], b1r)
            b2_sb = singles.tile([128, QC], F32)
            nc.sync.dma_start(b2_sb[:], b2r)
            w3_sb = singles.tile([128, QC, 4], BF16)
            nc.sync.dma_start(w3_sb[:], w3r)

            b12 = singles.tile([128, QC], F32)
            nc.vector.tensor_add(b12[:], b1_sb[:], b2_sb[:])

            # ---- hpbT[q, b] = (W1 @ h.T)[q, b] + b1[q] + b2[q], replicated
            #      along free (b cycles with period 32) into brep ----
            brep = singles.tile([128, QC, GR], F32)
            for qc in range(QC):
                ps = hpp.tile([128, B], F32, tag="hp")
                for kc in range(KC):
                    nc.tensor.matmul(
                        ps[:],
                        lhsT=w1t_sb[:, kc, 128 * qc : 128 * (qc + 1)],
                        rhs=ht_sb[:, kc, :],
                        start=(kc == 0),
                        stop=(kc == KC - 1),
                    )
                nc.vector.tensor_scalar_add(
                    brep[:, qc, 0:B], ps[:], scalar1=b12[:, qc : qc + 1]
                )
                w = B
                while w < GR:
                    nc.vector.tensor_copy(brep[:, qc, w : 2 * w], brep[:, qc, 0:w])
                    w *= 2

            # PE warm-up: redundant hp repetitions keep the HAM activity
            # window busy while the first v chunk streams in.
            for it in range(WARMUP_ITERS):
                wu_ps = hpp.tile([128, B], F32, tag="hp")
                for kc in range(KC):
                    nc.tensor.matmul(
                        wu_ps[:],
                        lhsT=w1t_sb[:, kc, 0:128],
                        rhs=ht_sb[:, kc, :],
                        start=(kc == 0),
                        stop=(kc == KC - 1),
                    )

            # exp(e) in scrambled flat order: group g occupies columns
            # [GR*g, GR*(g+1)), all on partition 0 (PSUM 1-partition reads
            # are only legal at partition 0).
            scram = singles.tile([1, R], F32)
            gsum = singles.tile([1, NG], F32)   # per-group partial row sums
            stot = singles.tile([1, 4], F32)
            rinv = singles.tile([1, 4], F32)

            def w3_stage(x_tile, g):
                # e = w3 . x, contracting q on partitions; lhsT is w3
                # replicated to M=4 columns, partition 0 of PSUM holds e.
                e_ps = epp.tile([4, GR], F32)
                for qc in range(QC):
                    nc.tensor.matmul(
                        e_ps[:],
                        lhsT=w3_sb[:, qc, :],
                        rhs=x_tile[:, qc, :],
                        start=(qc == 0),
                        stop=(qc == QC - 1),
                    )
                nc.scalar.activation(
                    scram[0:1, GR * g : GR * (g + 1)],
                    e_ps[0:1, :],
                    AF.Exp,
                    accum_out=gsum[0:1, g : g + 1],
                )
                # Output row i = g//4 completes with group 4i+3: scale by the
                # reciprocal row sum and write it out right away.
                if g % 4 == 3:
                    i = g // 4
                    nc.vector.reduce_sum(
                        stot[0:1, i : i + 1], gsum[0:1, 4 * i : 4 * i + 4],
                        axis=mybir.AxisListType.X,
                    )
                    nc.vector.reciprocal(rinv[0:1, i : i + 1], stot[0:1, i : i + 1])
                    sl = scram[0:1, F * i : F * (i + 1)]
                    if i % 2 == 0:
                        nc.vector.tensor_scalar_mul(
                            sl, sl, scalar1=rinv[0:1, i : i + 1]
                        )
                    else:
                        nc.scalar.mul(sl, sl, mul=rinv[0:1, i : i + 1])
                    nc.sync.dma_start(out[i : i + 1, :], scram[0:1, F * i : F * (i + 1)])

            pend = None
            for sg in range(NSG):
                vt_sb = vtp.tile([128, PC, SG_ROWS], BF16)
                nc.sync.dma_start(vt_sb[:], vt[sg].rearrange("pc pi f -> pi pc f"))
                for lg in range(GPSG):
                    g = sg * GPSG + lg
                    z = zp.tile([128, QC, GR], BF16)
                    x = xp.tile([128, QC, GR], BF16)
                    for qc in range(QC):
                        vp = vpp.tile([128, GR], F32)
                        for pc in range(PC):
                            nc.tensor.matmul(
                                vp[:],
                                lhsT=w2t_sb[:, pc, 128 * qc : 128 * (qc + 1)],
                                rhs=vt_sb[:, pc, GR * lg : GR * (lg + 1)],
                                start=(pc == 0),
                                stop=(pc == PC - 1),
                            )
                        nc.vector.tensor_add(z[:, qc, :], vp[:], brep[:, qc, :])
                    nc.scalar.activation(x[:], z[:], AF.Tanh)
                    if pend is not None:
                        w3_stage(*pend)
                    pend = (x, g)
            w3_stage(*pend)

    nc.compile()
    return nc


def make_in_maps(inputs):
    import ml_dtypes

    h = np.asarray(inputs["h"], dtype=np.float32)
    v = np.asarray(inputs["v"], dtype=np.float32)
    W1 = np.asarray(inputs["W1"], dtype=np.float32)
    b1 = np.asarray(inputs["b1"], dtype=np.float32)
    W2 = np.asarray(inputs["W2"], dtype=np.float32)
    b2 = np.asarray(inputs["b2"], dtype=np.float32)
    w3 = np.asarray(inputs["w3"], dtype=np.float32)

    ht = np.ascontiguousarray(h.T).astype(ml_dtypes.bfloat16)        # [H, B]
    w1t = np.ascontiguousarray(W1.T).astype(ml_dtypes.bfloat16)      # [H, P]
    w2t = np.ascontiguousarray(W2.T).astype(ml_dtypes.bfloat16)      # [P, P]
    b1r = np.ascontiguousarray(b1.reshape(QC, 128).T)                # [128, QC]
    b2r = np.ascontiguousarray(b2.reshape(QC, 128).T)                # [128, QC]
    w3r = np.ascontiguousarray(
        np.broadcast_to(w3.reshape(QC, 128).T[:, :, None], (128, QC, 4))
    ).astype(ml_dtypes.bfloat16)

    in_maps = []
    for c in range(NCORES):
        vs = v[:, c * FL : (c + 1) * FL, :]              # [B, FL, P]
        # [P, FL, B] -> row index r = f_local*B + b (f-major), then chunked
        # as [sg, pc, 128, SG_ROWS] so one DMA loads a whole super-group.
        vtc = vs.transpose(2, 1, 0).reshape(PC, 128, NSG, SG_ROWS)
        vtc = np.ascontiguousarray(vtc.transpose(2, 0, 1, 3)).astype(
            ml_dtypes.bfloat16
        )
        in_maps.append(
            {"vt": vtc, "w2t": w2t, "w1t": w1t, "ht": ht,
             "b1r": b1r, "b2r": b2r, "w3r": w3r}
        )
    return in_maps


_NC_CACHE = None


def kernel(**inputs) -> np.ndarray:
    global _NC_CACHE, LAST_RESULTS
    if _NC_CACHE is None:
        _NC_CACHE = build_nc()
    nc = _NC_CACHE
    in_maps = make_in_maps(inputs)
    res = run_bass_kernel_spmd(nc, in_maps, core_ids=list(range(NCORES)),
                               trace=TRACE)
    LAST_RESULTS = res
    outs = [np.asarray(res.results[c]["out"]) for c in range(NCORES)]
    return np.concatenate(outs, axis=0).astype(np.float32)  # [B, F]
